# revision 1
# baseline (speedup 1.0000x reference)
"""AttentionAvg kernel for 8 Trainium2 NeuronCores.

Reference computation (per batch b):
    q = x @ Wq^T + bq; k = x @ Wk^T + bk          (t, d)
    s = q @ k^T / sqrt(d)                          (t, t)
    s[:, j] = -1e9 where mask[j] == 0
    w = softmax(s, axis=-1)
    out[b] = sum_t x[t] * w[t, t]                  (d,)

Only the *diagonal* of the softmax is needed:
    w[t, t] = exp(s_tt) / sum_j exp(s_tj)   (row-max shift cancels; scores are
    O(1) so raw exp is safe in fp32, and masked entries underflow to exact 0).

Masked keys contribute exactly 0 to every row sum, and masked rows have
w[t, t] == 0, so both can be dropped: we gather the unmasked rows once and
run the whole pipeline on the compacted length Tg (~T/2), shrinking the
dominant (t, t, d) matmul ~4x.

Sharding: data-parallel over batch, one batch row per core (8 == 8).

Per-core device pipeline (Tile framework):
  1. indirect-DMA gather of unmasked x rows; PE-transpose -> XT [d, Tg]
  2. PE-transpose weights; QT/KT = W @ XT (+bias, Q pre-scaled by 1/sqrt(d))
  3. S chunks [128 q, 512 k] = QT^T @ KT in PSUM, plus a rank-1 matmul that
     adds the -1e9 column mask bias; ACT exp(PSUM)->SBUF with accum_out
     giving the row sums; diagonal extracted with an identity-mask
     tensor_tensor_reduce.  All stats land in partition layout.
  4. w = diag * (1/Z); out = sum_t w_t * x_t via accumulating PE matvec
     against re-gathered x rows.
"""

import math
import sys

import numpy as np

for _p in ("/opt/trn_rl_repo",):
    if _p not in sys.path:
        sys.path.insert(0, _p)

import ml_dtypes  # noqa: E402

import concourse.bass as bass  # noqa: E402
from concourse import bacc  # noqa: E402
import concourse.mybir as mybir  # noqa: E402
import concourse.tile as tile  # noqa: E402

B, T, D = 8, 4096, 768
P = 128
DT = D // P  # 6 contraction tiles
CH = 512  # free-dim chunk width (one PSUM bank of fp32)
NCORES = 8
SCALE = 1.0 / math.sqrt(D)

F32 = mybir.dt.float32
F32R = mybir.dt.float32r
BF16 = mybir.dt.bfloat16
I32 = mybir.dt.int32


def _chunks(n, width):
    out = []
    c0 = 0
    while c0 < n:
        w = min(width, n - c0)
        out.append((c0, w))
        c0 += w
    return out


def build_graph(nc, Tg, qk_bf16=True, min_count=0):
    """Emit the per-core graph for gathered/padded length Tg (multiple of P)."""
    JB = Tg // P
    chunks = _chunks(Tg, CH)
    IC = len(chunks)
    # chunks strictly below every batch's unmasked count hold no padded
    # columns on any core -> the -1e9 rank-1 mask matmul can be skipped
    mask_chunk0 = min_count // CH
    qk_dt = BF16 if qk_bf16 else F32

    x = nc.declare_dram_parameter("x", [T, D], F32, isOutput=False)
    xb = nc.declare_dram_parameter("xb", [T, D], qk_dt, isOutput=False)
    idx = nc.declare_dram_parameter("idx", [P, Tg // P], I32, isOutput=False)
    mb = nc.declare_dram_parameter("mb", [Tg], qk_dt, isOutput=False)
    wqb = nc.declare_dram_parameter("wqb", [D, D], qk_dt, isOutput=False)
    bq = nc.declare_dram_parameter("bq", [P, D // P], F32, isOutput=False)
    idf = nc.declare_dram_parameter("idf", [P, P], F32, isOutput=False)
    idb = nc.declare_dram_parameter("idb", [P, P], qk_dt, isOutput=False)
    wkb = nc.declare_dram_parameter("wkb", [D, D], qk_dt, isOutput=False)
    bk = nc.declare_dram_parameter("bk", [P, D // P], F32, isOutput=False)
    out = nc.declare_dram_parameter("out", [1, D], F32, isOutput=True)

    with tile.TileContext(nc) as tc:
        with (
            tc.tile_pool(name="singles", bufs=1) as singles,
            tc.tile_pool(name="xtile", bufs=(6 if Tg <= 3072 else 4)) as xtile,
            tc.tile_pool(name="spool", bufs=4) as spool,
            tc.tile_pool(name="stats", bufs=6) as stats,
            tc.tile_pool(name="ptr", bufs=2, space="PSUM") as ptr,
            tc.tile_pool(name="psS", bufs=4, space="PSUM") as psS,
            tc.tile_pool(name="psO", bufs=1, space="PSUM") as psO,
        ):
            # ---- resident tensors ----
            XT = singles.tile([P, DT, Tg], qk_dt, tag="XT")
            QT = singles.tile([P, DT, Tg], qk_dt, tag="QT")
            KT = singles.tile([P, DT, Tg], qk_dt, tag="KT")
            WqT = singles.tile([P, DT, D], qk_dt, tag="WqT")
            WkT = singles.tile([P, DT, D], qk_dt, tag="WkT")

            # idx first: every gather depends on it
            idx_sb = singles.tile([P, JB], I32, tag="idx_sb")
            nc.sync.dma_start(idx_sb, idx[:, :])
            ident_qk = singles.tile([P, P], qk_dt, tag="ident_qk")
            nc.sync.dma_start(ident_qk, idb[:, :])
            identity = singles.tile([P, P], F32, tag="ident")
            nc.sync.dma_start(identity, idf[:, :])

            ones_row = singles.tile([1, P], qk_dt, tag="ones_row")
            nc.vector.memset(ones_row, 1.0)
            # mask bias as a single free-dim row (rank-1 matmul rhs)
            mb_row = singles.tile([1, Tg], qk_dt, tag="mb_row")
            nc.sync.dma_start(mb_row, mb.rearrange("(o t) -> o t", o=1))

            # biases in partition layout [p, e_tile]; Q bias pre-scaled
            bq_sb = singles.tile([P, DT], F32, tag="bq_sb")
            nc.sync.dma_start(bq_sb, bq[:, :])
            bk_sb = singles.tile([P, DT], F32, tag="bk_sb")
            nc.sync.dma_start(bk_sb, bk[:, :])
            bqs = singles.tile([P, DT], F32, tag="bqs")
            nc.vector.tensor_scalar_mul(bqs, bq_sb, SCALE)

            # ---- weight transposes (DMA xbar, DRAM source): WT[d, e] ----
            for wi, (wsrc, wdst) in enumerate(((wqb, WqT), (wkb, WkT))):
                for et in range(DT):
                    eng = nc.sync if (et + wi) % 2 == 0 else nc.scalar
                    eng.dma_start_transpose(
                        wdst[:, :, et * P : (et + 1) * P],
                        wsrc[et * P : (et + 1) * P, :],
                    )

            # ---- wavefront: per chunk s, gather+transpose+QK(s), then all
            # S(ib, jc) with max(chunk(ib), jc) == s.  The S work grows with s
            # and back-fills the PE while gathers pace the supply. ----
            Zbig = singles.tile([P, JB, IC], F32, tag="Zbig")
            diag_cols = singles.tile([P, JB], F32, tag="diag_cols")
            HD = D // 2
            po1 = psO.tile([1, HD], F32, tag="po1")
            po2 = psO.tile([1, HD], F32, tag="po2")

            def emit_qk(c0, w):
                for tb in range(c0 // P, (c0 + w + P - 1) // P):
                    xgb = xtile.tile([P, D], qk_dt, tag="xgb")
                    nc.gpsimd.indirect_dma_start(
                        out=xgb,
                        out_offset=None,
                        in_=xb[:, :],
                        in_offset=bass.IndirectOffsetOnAxis(
                            ap=idx_sb[:, tb : tb + 1], axis=0
                        ),
                    )
                    for dt_i in range(DT):
                        pt = ptr.tile([P, P], qk_dt, tag="pt")
                        nc.tensor.transpose(
                            pt, xgb[:, dt_i * P : (dt_i + 1) * P], ident_qk
                        )
                        nc.vector.tensor_copy(
                            out=XT[:, dt_i, tb * P : (tb + 1) * P], in_=pt
                        )
                for et in range(DT):
                    for dst, wT, bias, scale in (
                        (QT, WqT, bqs, SCALE),
                        (KT, WkT, bk_sb, 1.0),
                    ):
                        ps = psS.tile([P, CH], F32, tag="psS")
                        for dt_i in range(DT):
                            nc.tensor.matmul(
                                ps[:, :w],
                                lhsT=wT[:, dt_i, et * P : (et + 1) * P],
                                rhs=XT[:, dt_i, c0 : c0 + w],
                                start=(dt_i == 0),
                                stop=(dt_i == DT - 1),
                            )
                        nc.scalar.activation(
                            out=dst[:, et, c0 : c0 + w],
                            in_=ps[:, :w],
                            func=mybir.ActivationFunctionType.Identity,
                            bias=bias[:, et : et + 1],
                            scale=scale,
                        )

            def emit_s(ib, jc):
                c0, w = chunks[jc]
                ps = psS.tile([P, CH], F32, tag="psS")
                need_mask = jc >= mask_chunk0
                if need_mask:
                    nc.tensor.matmul(
                        ps[:, :w],
                        lhsT=ones_row,
                        rhs=mb_row[:, c0 : c0 + w],
                        start=True,
                        stop=False,
                    )
                for et in range(DT):
                    nc.tensor.matmul(
                        ps[:, :w],
                        lhsT=QT[:, et, ib * P : (ib + 1) * P],
                        rhs=KT[:, et, c0 : c0 + w],
                        start=(et == 0 and not need_mask),
                        stop=(et == DT - 1),
                    )
                e_sb = spool.tile([P, CH], F32, tag="esb")
                nc.scalar.activation(
                    out=e_sb[:, :w],
                    in_=ps[:, :w],
                    func=mybir.ActivationFunctionType.Exp,
                    accum_out=Zbig[:, ib, jc : jc + 1],
                )
                dj = ib * P
                if c0 <= dj < c0 + w:
                    off = dj - c0
                    ed = spool.tile([P, P], F32, tag="ed")
                    nc.scalar.activation(
                        out=ed,
                        in_=ps[:, off : off + P],
                        func=mybir.ActivationFunctionType.Exp,
                    )
                    dsc = spool.tile([P, P], F32, tag="dsc")
                    nc.vector.tensor_mul(dsc, ed, identity)
                    nc.vector.reduce_sum(
                        diag_cols[:, ib : ib + 1], dsc, axis=mybir.AxisListType.X
                    )

            fin_n = [0]

            def emit_finalize(ib):
                z = stats.tile([P, 1], F32, tag="z")
                nc.vector.reduce_sum(z, Zbig[:, ib, :], axis=mybir.AxisListType.X)
                rz = stats.tile([P, 1], F32, tag="rz")
                nc.vector.reciprocal(rz, z)
                wcol = stats.tile([P, 1], F32, tag="wcol")
                nc.vector.tensor_mul(wcol, diag_cols[:, ib : ib + 1], rz)
                xg = xtile.tile([P, D], F32, tag="xg2")
                nc.gpsimd.indirect_dma_start(
                    out=xg,
                    out_offset=None,
                    in_=x[:, :],
                    in_offset=bass.IndirectOffsetOnAxis(
                        ap=idx_sb[:, ib : ib + 1], axis=0
                    ),
                )
                for po, sl in ((po1, slice(0, HD)), (po2, slice(HD, D))):
                    nc.tensor.matmul(
                        po,
                        lhsT=wcol,
                        rhs=xg[:, sl],
                        start=(fin_n[0] == 0),
                        stop=(fin_n[0] == JB - 1),
                    )
                fin_n[0] += 1

            last = len(chunks) - 1
            for s, (c0, w) in enumerate(chunks):
                emit_qk(c0, w)
                sb0 = c0 // P
                sb1 = (c0 + w + P - 1) // P
                # rows whose QT chunk just completed, against all ready columns
                for ib in range(sb0, sb1):
                    for jc in range(s + 1):
                        emit_s(ib, jc)
                    if s == last:
                        emit_finalize(ib)
                # earlier rows against the newly ready KT column chunk
                for ib in range(0, sb0):
                    emit_s(ib, s)
                    if s == last:
                        emit_finalize(ib)
            out_sb = singles.tile([1, D], F32, tag="out_sb")
            nc.vector.tensor_copy(out=out_sb[:, :HD], in_=po1)
            nc.vector.tensor_copy(out=out_sb[:, HD:], in_=po2)
            nc.sync.dma_start(out[:, :], out_sb)

    return nc


def prepare_host_inputs(inputs, mask):
    """Per-batch gather indices + padded mask bias; common padded length Tg."""
    idxs, counts = [], []
    for b in range(B):
        nz = np.nonzero(mask[b])[0].astype(np.int32)
        idxs.append(nz)
        counts.append(len(nz))
    Tg = max(max(counts), 1)
    Tg = ((Tg + P - 1) // P) * P
    idx_arr = np.zeros((B, Tg), np.int32)
    mb_arr = np.full((B, Tg), -1e9, np.float32)
    for b in range(B):
        n = counts[b]
        if n == 0:
            continue
        idx_arr[b, :n] = idxs[b]
        idx_arr[b, n:] = idxs[b][0]
        mb_arr[b, :n] = 0.0
    return Tg, idx_arr, mb_arr, counts


def kernel(inputs, mask, Wq_w, Wq_b, Wk_w, Wk_b, qk_bf16=True, _trace=False):
    from concourse.bass_utils import run_bass_kernel_spmd

    inputs = np.ascontiguousarray(inputs, np.float32)
    mask = np.asarray(mask)
    Tg, idx_arr, mb_arr, counts = prepare_host_inputs(inputs, mask)

    qk_dt = ml_dtypes.bfloat16 if qk_bf16 else np.float32
    nc = bacc.Bacc()
    build_graph(nc, Tg, qk_bf16=qk_bf16, min_count=min(counts) if min(counts) > 0 else 0)
    nc.compile()

    wqb = np.ascontiguousarray(Wq_w, np.float32).astype(qk_dt)
    wkb = np.ascontiguousarray(Wk_w, np.float32).astype(qk_dt)
    JB = Tg // P
    idf = np.eye(P, dtype=np.float32)
    idb = np.eye(P, dtype=np.float32).astype(qk_dt)
    bq2 = np.ascontiguousarray(
        np.asarray(Wq_b, np.float32).reshape(D // P, P).T
    )
    bk2 = np.ascontiguousarray(
        np.asarray(Wk_b, np.float32).reshape(D // P, P).T
    )
    in_maps = []
    for b in range(B):
        in_maps.append(
            {
                "x": inputs[b],
                "xb": inputs[b].astype(qk_dt),
                "idx": np.ascontiguousarray(idx_arr[b].reshape(JB, P).T),
                "mb": mb_arr[b].astype(qk_dt),
                "wqb": wqb,
                "bq": bq2,
                "wkb": wkb,
                "bk": bk2,
                "idf": idf,
                "idb": idb,
            }
        )

    res = run_bass_kernel_spmd(
        nc, in_maps, core_ids=list(range(NCORES)), trace=_trace
    )
    out = np.stack([res.results[b]["out"][0] for b in range(B)], axis=0)

    # degenerate all-masked batch: softmax over a constant row is uniform
    for b in range(B):
        if counts[b] == 0:
            out[b] = inputs[b].mean(axis=0)

    if _trace:
        return out, res
    return out



# revision 13
# speedup vs baseline: 1.6458x; 1.6458x over previous
"""AttentionAvg kernel for 8 Trainium2 NeuronCores — v2.

Reference (per batch b):
    q = x @ Wq^T + bq; k = x @ Wk^T + bk          (t, d)
    s = q @ k^T / sqrt(d);  s[:, j] = -1e9 where mask[j] == 0
    w = softmax(s, axis=-1);  out[b] = sum_t x[t] * w[t, t]

Only the softmax *diagonal* is needed.  Expanding the scores,
    s[q, k] = x_q^T A x_k + u[q] + v[k] + c,
    A = Wq^T Wk / sqrt(d),   v = Wk^T bq / sqrt(d),
and the row-constant terms u[q] + c cancel in
    w[t, t] = exp(s_tt) / sum_k exp(s_tk).
So ONE projection  Y = X A  with v folded in as the per-partition
activation bias (Y_q = A^T x_q + v  =>  Y_q . x_k = x_q^T A x_k + v[k])
replaces the two d x d Q/K projections of the naive pipeline.

Masked rows/keys are dropped by a HOST-side gather of the unmasked rows,
zero-padded to a multiple of 128 (Tg ~ T/2 for this mask, shrinking the
dominant (t, t, d) matmul ~4x).  Zero-padded COLUMNS contribute exactly
exp(0) = 1 to every row sum — corrected by subtracting n_pad from Z on
device.  Zero-padded ROWS get harmless finite weights that multiply
all-zero x rows in the final matvec.  The host also pre-transposes and
casts everything, so the device performs no gathers and no transposes;
XT arrives chunked by plain DMA and the PE can start almost immediately.

Per-core device pipeline (Tile framework):
  1. per k-chunk: YT[:, :, chunk] = A^T @ XT(chunk) + v     (PE + ACT)
  2. wavefront S(ib, jc) = YT(ib)^T @ XT(jc) in PSUM; ACT exp -> SBUF
     with accum_out row-sums into Zbig; the diagonal block is extracted
     with a fused tensor_tensor_reduce against an identity mask.
  3. w = diag * 1/(Z - n_pad); out += w^T @ X via accumulating PE matvec
     against host-gathered x rows (bf16), PSUM -> SBUF -> DRAM.

Sharding: data-parallel over batch, one batch row per core (8 == 8).
"""

import math
import sys

import numpy as np

for _p in ("/opt/trn_rl_repo",):
    if _p not in sys.path:
        sys.path.insert(0, _p)

import ml_dtypes  # noqa: E402

import concourse.bass as bass  # noqa: E402
from concourse import bacc  # noqa: E402
import concourse.mybir as mybir  # noqa: E402
import concourse.tile as tile  # noqa: E402

B, T, D = 8, 4096, 768
P = 128
DT = D // P  # 6 contraction tiles
CH = 512  # free-dim chunk width (one PSUM bank of fp32)
HD = D // 2  # finalize matvec split (<=512 per PSUM bank)
NCORES = 8
SCALE = 1.0 / math.sqrt(D)

F32 = mybir.dt.float32
BF16 = mybir.dt.bfloat16
BF = ml_dtypes.bfloat16


def _chunks(n, width):
    """Remainder-FIRST chunking: smallest chunk leads, shrinking the
    DMA->first-matmul head latency."""
    out = []
    c0 = 0
    rem = n % width
    if rem:
        out.append((0, rem))
        c0 = rem
    while c0 < n:
        out.append((c0, width))
        c0 += width
    return out


def build_graph(nc, Tg):
    """Emit the per-core graph for gathered/padded length Tg (multiple of P)."""
    JB = Tg // P
    chunks = _chunks(Tg, CH)

    IC = len(chunks)

    xt = nc.declare_dram_parameter("xt", [DT, P, Tg], BF16, isOutput=False)
    aw = nc.declare_dram_parameter("aw", [DT, P, D], BF16, isOutput=False)
    vb = nc.declare_dram_parameter("vb", [P, DT], F32, isOutput=False)
    idf = nc.declare_dram_parameter("idf", [P, P], F32, isOutput=False)
    npz = nc.declare_dram_parameter("npz", [P, JB], F32, isOutput=False)
    xg = nc.declare_dram_parameter("xg", [JB, P, D], BF16, isOutput=False)
    out = nc.declare_dram_parameter("out", [1, D], F32, isOutput=True)

    with tile.TileContext(nc) as tc:
        with (
            tc.tile_pool(name="singles", bufs=1) as singles,
            tc.tile_pool(name="spool", bufs=4) as spool,
            tc.tile_pool(name="stats", bufs=6) as stats,
            tc.tile_pool(name="psS", bufs=6, space="PSUM") as psS,
            tc.tile_pool(name="psO", bufs=1, space="PSUM") as psO,
        ):
            # ---- resident tensors ----
            XT = singles.tile([P, DT, Tg], BF16, tag="XT")
            YT = singles.tile([P, DT, Tg], BF16, tag="YT")
            XG = singles.tile([P, JB, D], BF16, tag="XG")
            AW = singles.tile([P, DT, D], BF16, tag="AW")
            VB = singles.tile([P, DT], F32, tag="VB")
            identity = singles.tile([P, P], F32, tag="ident")
            # [jc, ib] layout; the extra IC row holds -n_pad so the plain
            # row-sum over jc comes out already pad-corrected
            Zbig = singles.tile([P, IC + 1, JB], F32, tag="Zbig")
            diag_cols = singles.tile([P, JB], F32, tag="diag_cols")

            # ---- DMA issue: A first (needed by every Y matmul), then XT
            # chunks round-robin over queues, small singles, then xg rows
            # (only needed at finalize). ----
            for ei in range(DT):
                nc.sync.dma_start(AW[:, ei, :], aw[ei, :, :])
            qs = (nc.sync, nc.scalar)
            qi = 0
            for c0, w in chunks:
                for ei in range(DT):
                    qs[qi % len(qs)].dma_start(
                        XT[:, ei, c0 : c0 + w], xt[ei, :, c0 : c0 + w]
                    )
                    qi += 1
            nc.scalar.dma_start(VB, vb[:, :])
            nc.scalar.dma_start(Zbig[:, IC, :], npz[:, :])
            nc.scalar.dma_start(identity, idf[:, :])
            for ib in range(JB):
                qs[ib % len(qs)].dma_start(XG[:, ib, :], xg[ib, :, :])

            po1 = psO.tile([1, HD], F32, tag="po1")
            po2 = psO.tile([1, HD], F32, tag="po2")

            def emit_y(c0, w):
                for eo in range(DT):
                    ps = psS.tile([P, CH], F32, tag="psS")
                    for ei in range(DT):
                        nc.tensor.matmul(
                            ps[:, :w],
                            lhsT=AW[:, ei, eo * P : (eo + 1) * P],
                            rhs=XT[:, ei, c0 : c0 + w],
                            start=(ei == 0),
                            stop=(ei == DT - 1),
                        )
                    nc.scalar.activation(
                        out=YT[:, eo, c0 : c0 + w],
                        in_=ps[:, :w],
                        func=mybir.ActivationFunctionType.Identity,
                        bias=VB[:, eo : eo + 1],
                        scale=1.0,
                    )

            def emit_s(ib, jc):
                c0, w = chunks[jc]
                ps = psS.tile([P, CH], F32, tag="psS")
                for et in range(DT):
                    nc.tensor.matmul(
                        ps[:, :w],
                        lhsT=YT[:, et, ib * P : (ib + 1) * P],
                        rhs=XT[:, et, c0 : c0 + w],
                        start=(et == 0),
                        stop=(et == DT - 1),
                    )
                e_sb = spool.tile([P, CH], F32, tag="esb")
                nc.scalar.activation(
                    out=e_sb[:, :w],
                    in_=ps[:, :w],
                    func=mybir.ActivationFunctionType.Exp,
                    accum_out=Zbig[:, jc, ib : ib + 1],
                )
                dj = ib * P
                if c0 <= dj < c0 + w:
                    off = dj - c0
                    dsc = spool.tile([P, P], F32, tag="dsc")
                    nc.vector.tensor_mul(dsc, e_sb[:, off : off + P], identity)
                    nc.vector.reduce_sum(
                        diag_cols[:, ib : ib + 1], dsc, axis=mybir.AxisListType.X
                    )

            fin_n = [0]

            def emit_finalize(ib):
                z = stats.tile([P, 1], F32, tag="z")
                nc.vector.reduce_sum(
                    z, Zbig[:, :, ib : ib + 1], axis=mybir.AxisListType.XY
                )
                rz = stats.tile([P, 1], F32, tag="rz")
                nc.vector.reciprocal(rz, z)
                wcol = stats.tile([P, 1], BF16, tag="wcol")
                nc.vector.tensor_mul(wcol, diag_cols[:, ib : ib + 1], rz)
                for po, sl in ((po1, slice(0, HD)), (po2, slice(HD, D))):
                    nc.tensor.matmul(
                        po,
                        lhsT=wcol,
                        rhs=XG[:, ib, sl],
                        start=(fin_n[0] == 0),
                        stop=(fin_n[0] == JB - 1),
                    )
                fin_n[0] += 1

            # ---- wavefront: per chunk s compute YT(s), then all S(ib, jc)
            # with max(block(ib), jc) == s ----
            last = len(chunks) - 1
            for s, (c0, w) in enumerate(chunks):
                emit_y(c0, w)
                sb0 = c0 // P
                sb1 = (c0 + w + P - 1) // P
                for ib in range(sb0, sb1):
                    for jc in range(s + 1):
                        emit_s(ib, jc)
                    if s == last:
                        emit_finalize(ib)
                for ib in range(0, sb0):
                    emit_s(ib, s)
                    if s == last:
                        emit_finalize(ib)

            out_sb = singles.tile([1, D], F32, tag="out_sb")
            nc.vector.tensor_copy(out=out_sb[:, :HD], in_=po1)
            nc.vector.tensor_copy(out=out_sb[:, HD:], in_=po2)
            nc.sync.dma_start(out[:, :], out_sb)

    return nc


def prepare_host_inputs(inputs, mask):
    """Per-batch gather + zero-pad to the common padded length Tg."""
    idxs, counts = [], []
    for b in range(B):
        nz = np.nonzero(mask[b])[0]
        idxs.append(nz)
        counts.append(len(nz))
    Tg = max(max(counts), 1)
    Tg = ((Tg + P - 1) // P) * P
    return Tg, idxs, counts


def kernel(inputs, mask, Wq_w, Wq_b, Wk_w, Wk_b, qk_bf16=True, _trace=False):
    from concourse.bass_utils import run_bass_kernel_spmd

    inputs = np.ascontiguousarray(inputs, np.float32)
    mask = np.asarray(mask)
    Tg, idxs, counts = prepare_host_inputs(inputs, mask)
    JB = Tg // P

    nc = bacc.Bacc()
    build_graph(nc, Tg)
    nc.compile()

    # s * Wq^T Wk  and  s * Wk^T bq  (row-constant score terms cancel)
    A = (np.asarray(Wq_w, np.float32).T @ np.asarray(Wk_w, np.float32)) * SCALE
    vvec = (np.asarray(Wk_w, np.float32).T @ np.asarray(Wq_b, np.float32)) * SCALE
    aw_arr = np.ascontiguousarray(A.astype(BF).reshape(DT, P, D))
    vb_arr = np.ascontiguousarray(vvec.reshape(DT, P).T)
    idf = np.eye(P, dtype=np.float32)

    in_maps = []
    for b in range(B):
        n = counts[b]
        xg_f = np.zeros((Tg, D), np.float32)
        if n:
            xg_f[:n] = inputs[b][idxs[b]]
        xg_bf = xg_f.astype(BF)
        xt_arr = np.ascontiguousarray(xg_bf.T.reshape(DT, P, Tg))
        xg_arr = xg_bf.reshape(JB, P, D)
        in_maps.append(
            {
                "xt": xt_arr,
                "aw": aw_arr,
                "vb": vb_arr,
                "idf": idf,
                "npz": np.full((P, JB), -float(Tg - n), np.float32),
                "xg": xg_arr,
            }
        )

    res = run_bass_kernel_spmd(
        nc, in_maps, core_ids=list(range(NCORES)), trace=_trace
    )
    out = np.stack([res.results[b]["out"][0] for b in range(B)], axis=0)

    # degenerate all-masked batch: softmax over a constant row is uniform
    for b in range(B):
        if counts[b] == 0:
            out[b] = inputs[b].mean(axis=0)

    if _trace:
        return out, res
    return out
